# revision 8
# baseline (speedup 1.0000x reference)
"""Trainium2 Bass kernel for a binarized BasicBlock (BinConv3x3 + scale + sync-BN + residual).

Reference computation (NCHW, N=64, C=256, H=W=28):
    out = BN_train(scale * conv3x3(sign(x), sign(w))) + x

Strategy: data-parallel over batch across 8 NeuronCores (8 images/core).
  - host: binarize weights (sign -> bf16 lhsT tiles), fold gamma/scale/beta
  - device per core:
      sign(x) -> zero-padded bf16 tiles [128cin, 30, 30] per (image, cin-block)
      conv as 9 shifted matmuls x 2 cin-blocks accumulated in PSUM
      PSUM evacuation on ScalarE with row-accumulate -> per-channel sum(z), sum(z^2)
      2KB AllReduce of the partial sums across the 8 cores (exact sync-BN)
      per-channel A,B finalization; apply A*z+B + x on ScalarE/VectorE; DMA out
"""

import os
import sys

sys.path.insert(0, "/opt/trn_rl_repo")

import numpy as np
import ml_dtypes

import concourse.bass as bass
import concourse.mybir as mybir
import concourse.tile as tile
from concourse import bacc
from concourse.bass_utils import run_bass_kernel_spmd

AF = mybir.ActivationFunctionType
ALU = mybir.AluOpType

N_CORES = 8
N_PER_CORE = 8          # images per core
C = 256                 # channels
CB = 2                  # channel blocks of 128
P = 128                 # partitions
H = W = 28
HW = H * W              # 784
HP = WP = 30            # padded spatial
HALF = 14               # output rows per matmul group
NFREE = HALF * W        # 392 free elems per matmul
BN_EPS = 1e-5
N_TOTAL_ELEMS = 64 * HW  # BN normalizer: N*H*W over the full batch

_CACHED = None


def _build_nc():
    nc = bacc.Bacc("TRN2", target_bir_lowering=False, debug=False,
                   num_devices=N_CORES)

    x_dram = nc.dram_tensor("x", [N_PER_CORE, CB, P, HW], mybir.dt.float32,
                            kind="ExternalInput")
    wb_dram = nc.dram_tensor("wb", [CB * 9 * CB, P, P], mybir.dt.bfloat16,
                             kind="ExternalInput")
    pp_dram = nc.dram_tensor("pp", [P, CB, 3], mybir.dt.float32,
                             kind="ExternalInput")
    out_dram = nc.dram_tensor("out", [N_PER_CORE, CB, P, HW], mybir.dt.float32,
                              kind="ExternalOutput")

    NW = CB * 9 * CB  # 36 weight tiles

    with tile.TileContext(nc) as tc:
        with (
            tc.tile_pool(name="const", bufs=1) as cpool,
            tc.tile_pool(name="xin", bufs=1) as xpool,
            tc.tile_pool(name="spad", bufs=1) as spool,
            tc.tile_pool(name="z", bufs=1) as zpool,
            tc.tile_pool(name="sq", bufs=2) as sqpool,
            tc.tile_pool(name="small", bufs=1) as mpool,
            tc.tile_pool(name="psum", bufs=8, space="PSUM") as psum,
            tc.tile_pool(name="dram", bufs=1, space="DRAM") as dram,
        ):
            wt = cpool.tile([P, NW, P], mybir.dt.bfloat16)
            nc.sync.dma_start(wt[:], wb_dram.ap().rearrange("i k m -> k i m"))

            pp = cpool.tile([P, CB, 3], mybir.dt.float32)
            nc.sync.dma_start(pp[:], pp_dram[:])

            # per-(image, cin-block) input and padded-sign tiles
            xts = []
            sts = []
            for t in range(N_PER_CORE * CB):
                st = spool.tile([P, HP, WP], mybir.dt.bfloat16,
                                name=f"spad{t}", tag=f"spad{t}")
                nc.vector.memset(st[:], 0.0)
                sts.append(st)

            for n in range(N_PER_CORE):
                for cb in range(CB):
                    t = n * CB + cb
                    xt = xpool.tile([P, HW], mybir.dt.float32,
                                    name=f"xin{t}", tag=f"xin{t}")
                    nc.sync.dma_start(xt[:], x_dram[n, cb])
                    xts.append(xt)
                    # sign(x) into the interior of the zero-padded tile
                    nc.scalar.activation(sts[t][:, 1:29, 1:29], xt[:], AF.Sign)

            # conv output, raw (unscaled) integer-valued sums
            z = zpool.tile([P, CB, N_PER_CORE, HW], mybir.dt.float32)
            # per-chunk row-sum partials (one column per psum tile)
            s1c = mpool.tile([P, CB, 2 * N_PER_CORE], mybir.dt.float32)
            s2c = mpool.tile([P, CB, 2 * N_PER_CORE], mybir.dt.float32)

            def widx(cob, tap, cib):
                return (cob * 9 + tap) * CB + cib

            # Conv: 4 PSUM tiles (2 images x 2 halves) accumulate per weight
            # load so each lhsT is reused 4x and PE streams without gaps.
            for g in range(N_PER_CORE // 2):        # image pair
                for cob in range(CB):
                    pss = [psum.tile([P, NFREE], mybir.dt.float32,
                                     name=f"ps_{g}_{cob}_{j}", tag="ps")
                           for j in range(4)]
                    for cib in range(CB):
                        for dh in range(3):
                            for dw in range(3):
                                w_ap = wt[:, widx(cob, dh * 3 + dw, cib), :]
                                first = (cib == 0 and dh == 0 and dw == 0)
                                last = (cib == 1 and dh == 2 and dw == 2)
                                for j in range(4):
                                    n = g * 2 + j // 2
                                    h0 = (j % 2) * HALF
                                    nc.tensor.matmul(
                                        pss[j][:],
                                        w_ap,
                                        sts[n * CB + cib][:, h0 + dh:h0 + dh + HALF,
                                                          dw:dw + W],
                                        start=first,
                                        stop=last,
                                    )
                    for j in range(4):
                        n = g * 2 + j // 2
                        half = j % 2
                        h0 = half * HALF
                        idx = n * 2 + half
                        zsl = z[:, cob, n, h0 * W:(h0 + HALF) * W]
                        nc.scalar.activation(
                            zsl, pss[j][:],
                            AF.Copy, accum_out=s1c[:, cob, idx:idx + 1])
                        sq = sqpool.tile([P, NFREE], mybir.dt.float32, tag="sq")
                        nc.vector.tensor_mul(sq[:], zsl, zsl)
                        nc.vector.tensor_reduce(
                            s2c[:, cob, idx:idx + 1], sq[:],
                            axis=mybir.AxisListType.X, op=ALU.add)

            # local stats -> [128, 4] = [s1_b0, s1_b1, s2_b0, s2_b1]
            cc_sb = mpool.tile([P, 4], mybir.dt.float32)
            nc.vector.tensor_reduce(cc_sb[:, 0:2], s1c[:],
                                    axis=mybir.AxisListType.X, op=ALU.add)
            nc.vector.tensor_reduce(cc_sb[:, 2:4], s2c[:],
                                    axis=mybir.AxisListType.X, op=ALU.add)

            # exact sync-BN: AllReduce the 2KB of partial sums
            cc_in = dram.tile([P, 4], mybir.dt.float32)
            cc_out = dram.tile([P, 4], mybir.dt.float32, addr_space="Shared")
            nc.gpsimd.dma_start(cc_in[:], cc_sb[:])
            nc.gpsimd.collective_compute(
                "AllReduce", ALU.add,
                replica_groups=[list(range(N_CORES))],
                ins=[cc_in[:]],
                outs=[cc_out[:]],
            )
            tot = mpool.tile([P, 4], mybir.dt.float32)
            nc.gpsimd.dma_start(tot[:], cc_out[:])

            # per-channel finalization:
            #   mu_z = S1/M ; var_z = S2/M - mu_z^2 ; var_y = scale^2*var_z
            #   A = gamma*scale/sqrt(var_y+eps) ; B = beta - A*mu_z
            inv = 1.0 / N_TOTAL_ELEMS
            mu = mpool.tile([P, CB], mybir.dt.float32)
            ez2 = mpool.tile([P, CB], mybir.dt.float32)
            m2 = mpool.tile([P, CB], mybir.dt.float32)
            varz = mpool.tile([P, CB], mybir.dt.float32)
            vary = mpool.tile([P, CB], mybir.dt.float32)
            stdv = mpool.tile([P, CB], mybir.dt.float32)
            rstd = mpool.tile([P, CB], mybir.dt.float32)
            A = mpool.tile([P, CB], mybir.dt.float32)
            t0 = mpool.tile([P, CB], mybir.dt.float32)
            B = mpool.tile([P, CB], mybir.dt.float32)

            nc.vector.tensor_scalar_mul(mu[:], tot[:, 0:2], inv)
            nc.vector.tensor_scalar_mul(ez2[:], tot[:, 2:4], inv)
            nc.vector.tensor_mul(m2[:], mu[:], mu[:])
            nc.vector.tensor_sub(varz[:], ez2[:], m2[:])
            nc.vector.tensor_mul(vary[:], varz[:], pp[:, :, 0])
            nc.vector.tensor_scalar_add(vary[:], vary[:], BN_EPS)
            nc.scalar.activation(stdv[:], vary[:], AF.Sqrt)
            nc.vector.reciprocal(rstd[:], stdv[:])
            nc.vector.tensor_mul(A[:], rstd[:], pp[:, :, 1])
            nc.vector.tensor_mul(t0[:], A[:], mu[:])
            nc.vector.tensor_sub(B[:], pp[:, :, 2], t0[:])

            # apply: out = A*z + B + x, then DMA out
            for n in range(N_PER_CORE):
                for cb in range(CB):
                    zs = z[:, cb, n, :]
                    nc.scalar.activation(zs, zs, AF.Identity,
                                         scale=A[:, cb:cb + 1],
                                         bias=B[:, cb:cb + 1])
                    nc.vector.tensor_add(zs, zs, xts[n * CB + cb][:])
                    nc.sync.dma_start(out_dram[n, cb], zs)

    nc.compile()
    return nc


def _prep_shared(w, scale, gamma, beta):
    w = np.asarray(w, dtype=np.float32)
    scale = np.asarray(scale, dtype=np.float32).reshape(C)
    gamma = np.asarray(gamma, dtype=np.float32).reshape(C)
    beta = np.asarray(beta, dtype=np.float32).reshape(C)

    wsign = np.sign(w).astype(ml_dtypes.bfloat16)
    # lhsT[cob, dh, dw, cib, k, m] = wsign[cob*128+m, cib*128+k, dh, dw]
    arr = wsign.reshape(CB, P, CB, P, 3, 3).transpose(0, 4, 5, 2, 3, 1)
    wb = np.ascontiguousarray(arr.reshape(CB * 9 * CB, P, P))

    pp = np.empty((P, CB, 3), dtype=np.float32)
    for cb in range(CB):
        ch = slice(cb * P, (cb + 1) * P)
        pp[:, cb, 0] = scale[ch] * scale[ch]
        pp[:, cb, 1] = gamma[ch] * scale[ch]
        pp[:, cb, 2] = beta[ch]
    return wb, pp


def kernel(x, w, scale, gamma, beta):
    global _CACHED
    if _CACHED is None:
        _CACHED = _build_nc()
    nc = _CACHED

    x = np.asarray(x, dtype=np.float32)
    wb, pp = _prep_shared(w, scale, gamma, beta)

    in_maps = []
    for i in range(N_CORES):
        xs = x[i * N_PER_CORE:(i + 1) * N_PER_CORE]
        xs = np.ascontiguousarray(xs.reshape(N_PER_CORE, CB, P, HW))
        in_maps.append({"x": xs, "wb": wb, "pp": pp})

    trace = bool(int(os.environ.get("KERNEL_TRACE", "0")))
    kw = {}
    tdir = os.environ.get("KERNEL_TRACE_DIR")
    if trace and tdir:
        os.makedirs(tdir, exist_ok=True)
        kw["tmpdir"] = tdir
    res = run_bass_kernel_spmd(nc, in_maps, core_ids=list(range(N_CORES)),
                               trace=trace, **kw)
    if trace:
        import kernel as _self
        _self.LAST_EXEC_NS = res.exec_time_ns
        _self.LAST_RESULTS = res

    out = np.empty((64, C, H, W), dtype=np.float32)
    for i in range(N_CORES):
        o = res.results[i]["out"].reshape(N_PER_CORE, C, H, W)
        out[i * N_PER_CORE:(i + 1) * N_PER_CORE] = o
    return out


# revision 13
# speedup vs baseline: 1.0300x; 1.0300x over previous
"""Trainium2 Bass kernel for a binarized BasicBlock (BinConv3x3 + scale + sync-BN + residual).

Reference computation (NCHW, N=64, C=256, H=W=28):
    out = BN_train(scale * conv3x3(sign(x), sign(w))) + x

Strategy: data-parallel over batch across 8 NeuronCores (8 images/core).
  - host: binarize weights (sign -> bf16 lhsT tiles), fold gamma/scale/beta
  - device per core:
      sign(x) -> zero-padded bf16 tiles [128cin, 30, 30] per (image, cin-block)
      conv as 9 shifted matmuls x 2 cin-blocks accumulated in PSUM
      PSUM evacuation on ScalarE with row-accumulate -> per-channel sum(z), sum(z^2)
      2KB AllReduce of the partial sums across the 8 cores (exact sync-BN)
      per-channel A,B finalization; apply A*z+B + x on ScalarE/VectorE; DMA out
"""

import os
import sys

sys.path.insert(0, "/opt/trn_rl_repo")

import numpy as np
import ml_dtypes

import concourse.bass as bass
import concourse.mybir as mybir
import concourse.tile as tile
from concourse import bacc
from concourse.bass_utils import run_bass_kernel_spmd

AF = mybir.ActivationFunctionType
ALU = mybir.AluOpType

N_CORES = 8
N_PER_CORE = 8          # images per core
C = 256                 # channels
CB = 2                  # channel blocks of 128
P = 128                 # partitions
H = W = 28
HW = H * W              # 784
HP = WP = 30            # padded spatial
HALF = 14               # output rows per matmul group
NFREE = HALF * W        # 392 free elems per matmul
BN_EPS = 1e-5
N_TOTAL_ELEMS = 64 * HW  # BN normalizer: N*H*W over the full batch

_CACHED = None


def _build_nc():
    nc = bacc.Bacc("TRN2", target_bir_lowering=False, debug=False,
                   num_devices=N_CORES)

    x_dram = nc.dram_tensor("x", [N_PER_CORE, CB, P, HW], mybir.dt.float32,
                            kind="ExternalInput")
    wb_dram = nc.dram_tensor("wb", [P, CB * 9 * CB, P], mybir.dt.bfloat16,
                             kind="ExternalInput")
    pp_dram = nc.dram_tensor("pp", [P, CB, 3], mybir.dt.float32,
                             kind="ExternalInput")
    out_dram = nc.dram_tensor("out", [N_PER_CORE, CB, P, HW], mybir.dt.float32,
                              kind="ExternalOutput")

    NW = CB * 9 * CB  # 36 weight tiles

    with tile.TileContext(nc) as tc:
        with (
            tc.tile_pool(name="const", bufs=1) as cpool,
            tc.tile_pool(name="xin", bufs=1) as xpool,
            tc.tile_pool(name="spad", bufs=1) as spool,
            tc.tile_pool(name="z", bufs=1) as zpool,
            tc.tile_pool(name="sq", bufs=2) as sqpool,
            tc.tile_pool(name="small", bufs=1) as mpool,
            tc.tile_pool(name="psum", bufs=8, space="PSUM") as psum,
            tc.tile_pool(name="dram", bufs=1, space="DRAM") as dram,
        ):
            wt = cpool.tile([P, NW, P], mybir.dt.bfloat16)
            nc.sync.dma_start(wt[:], wb_dram[:])

            pp = cpool.tile([P, CB, 3], mybir.dt.float32)
            nc.sync.dma_start(pp[:], pp_dram[:])

            # per-(image, cin-block) input and padded-sign tiles
            xts = []
            sts = []
            for t in range(N_PER_CORE * CB):
                st = spool.tile([P, HP, WP], mybir.dt.bfloat16,
                                name=f"spad{t}", tag=f"spad{t}")
                nc.vector.memset(st[:], 0.0)
                sts.append(st)

            for n in range(N_PER_CORE):
                for cb in range(CB):
                    t = n * CB + cb
                    xt = xpool.tile([P, HW], mybir.dt.float32,
                                    name=f"xin{t}", tag=f"xin{t}")
                    nc.sync.dma_start(xt[:], x_dram[n, cb])
                    xts.append(xt)
                    # sign(x) into the interior of the zero-padded tile
                    nc.scalar.activation(sts[t][:, 1:29, 1:29], xt[:], AF.Sign)

            # conv output, raw (unscaled) integer-valued sums
            z = zpool.tile([P, CB, N_PER_CORE, HW], mybir.dt.float32)
            # per-chunk row-sum partials (one column per psum tile)
            s1c = mpool.tile([P, CB, 2 * N_PER_CORE], mybir.dt.float32)
            s2c = mpool.tile([P, CB, 2 * N_PER_CORE], mybir.dt.float32)

            def widx(cob, tap, cib):
                return (cob * 9 + tap) * CB + cib

            # Conv: 4 PSUM tiles (2 images x 2 halves) accumulate per weight
            # load so each lhsT is reused 4x and PE streams without gaps.
            for g in range(N_PER_CORE // 2):        # image pair
                for cob in range(CB):
                    pss = [psum.tile([P, NFREE], mybir.dt.float32,
                                     name=f"ps_{g}_{cob}_{j}", tag="ps")
                           for j in range(4)]
                    for cib in range(CB):
                        for dh in range(3):
                            for dw in range(3):
                                w_ap = wt[:, widx(cob, dh * 3 + dw, cib), :]
                                first = (cib == 0 and dh == 0 and dw == 0)
                                last = (cib == 1 and dh == 2 and dw == 2)
                                for j in range(4):
                                    n = g * 2 + j // 2
                                    h0 = (j % 2) * HALF
                                    nc.tensor.matmul(
                                        pss[j][:],
                                        w_ap,
                                        sts[n * CB + cib][:, h0 + dh:h0 + dh + HALF,
                                                          dw:dw + W],
                                        start=first,
                                        stop=last,
                                    )
                    for j in range(4):
                        n = g * 2 + j // 2
                        half = j % 2
                        h0 = half * HALF
                        idx = n * 2 + half
                        zsl = z[:, cob, n, h0 * W:(h0 + HALF) * W]
                        nc.scalar.activation(
                            zsl, pss[j][:],
                            AF.Copy, accum_out=s1c[:, cob, idx:idx + 1])
                        sq = sqpool.tile([P, NFREE], mybir.dt.float32, tag="sq")
                        nc.vector.tensor_mul(sq[:], zsl, zsl)
                        nc.vector.tensor_reduce(
                            s2c[:, cob, idx:idx + 1], sq[:],
                            axis=mybir.AxisListType.X, op=ALU.add)

            # local stats -> [128, 4] = [s1_b0, s1_b1, s2_b0, s2_b1]
            cc_sb = mpool.tile([P, 4], mybir.dt.float32)
            nc.vector.tensor_reduce(cc_sb[:, 0:2], s1c[:],
                                    axis=mybir.AxisListType.X, op=ALU.add)
            nc.vector.tensor_reduce(cc_sb[:, 2:4], s2c[:],
                                    axis=mybir.AxisListType.X, op=ALU.add)

            # exact sync-BN: AllGather the 2KB of partial sums (lower latency
            # than AllReduce), then reduce the 8 rank contributions locally.
            cc_in = dram.tile([P, 4], mybir.dt.float32)
            ag_out = dram.tile([N_CORES, P, 4], mybir.dt.float32,
                               addr_space="Shared")
            nc.gpsimd.dma_start(cc_in[:], cc_sb[:])
            nc.gpsimd.collective_compute(
                "AllGather", ALU.bypass,
                replica_groups=[list(range(N_CORES))],
                ins=[cc_in[:]],
                outs=[ag_out[:]],
            )
            tot8 = mpool.tile([P, 4, N_CORES], mybir.dt.float32)
            nc.gpsimd.dma_start(tot8[:], ag_out[:].rearrange("r p c -> p c r"))
            tot = mpool.tile([P, 4], mybir.dt.float32)
            nc.vector.tensor_reduce(tot[:], tot8[:],
                                    axis=mybir.AxisListType.X, op=ALU.add)

            # per-channel finalization:
            #   mu_z = S1/M ; var_z = S2/M - mu_z^2 ; var_y = scale^2*var_z
            #   A = gamma*scale/sqrt(var_y+eps) ; B = beta - A*mu_z
            inv = 1.0 / N_TOTAL_ELEMS
            mu = mpool.tile([P, CB], mybir.dt.float32)
            ez2 = mpool.tile([P, CB], mybir.dt.float32)
            m2 = mpool.tile([P, CB], mybir.dt.float32)
            varz = mpool.tile([P, CB], mybir.dt.float32)
            vary = mpool.tile([P, CB], mybir.dt.float32)
            stdv = mpool.tile([P, CB], mybir.dt.float32)
            rstd = mpool.tile([P, CB], mybir.dt.float32)
            A = mpool.tile([P, CB], mybir.dt.float32)
            t0 = mpool.tile([P, CB], mybir.dt.float32)
            B = mpool.tile([P, CB], mybir.dt.float32)

            nc.vector.tensor_scalar_mul(mu[:], tot[:, 0:2], inv)
            nc.vector.tensor_scalar_mul(ez2[:], tot[:, 2:4], inv)
            nc.vector.tensor_mul(m2[:], mu[:], mu[:])
            nc.vector.tensor_sub(varz[:], ez2[:], m2[:])
            nc.vector.tensor_mul(vary[:], varz[:], pp[:, :, 0])
            nc.vector.tensor_scalar_add(vary[:], vary[:], BN_EPS)
            nc.scalar.activation(stdv[:], vary[:], AF.Sqrt)
            nc.vector.reciprocal(rstd[:], stdv[:])
            nc.vector.tensor_mul(A[:], rstd[:], pp[:, :, 1])
            nc.vector.tensor_mul(t0[:], A[:], mu[:])
            nc.vector.tensor_sub(B[:], pp[:, :, 2], t0[:])

            # apply: out = A*z + B + x, then DMA out
            for n in range(N_PER_CORE):
                for cb in range(CB):
                    zs = z[:, cb, n, :]
                    nc.scalar.activation(zs, zs, AF.Identity,
                                         scale=A[:, cb:cb + 1],
                                         bias=B[:, cb:cb + 1])
                    nc.vector.tensor_add(zs, zs, xts[n * CB + cb][:])
                    nc.sync.dma_start(out_dram[n, cb], zs)

    nc.compile()
    return nc


def _prep_shared(w, scale, gamma, beta):
    w = np.asarray(w, dtype=np.float32)
    scale = np.asarray(scale, dtype=np.float32).reshape(C)
    gamma = np.asarray(gamma, dtype=np.float32).reshape(C)
    beta = np.asarray(beta, dtype=np.float32).reshape(C)

    wsign = np.sign(w).astype(ml_dtypes.bfloat16)
    # lhsT[cob, dh, dw, cib, k, m] = wsign[cob*128+m, cib*128+k, dh, dw];
    # stored [k, idx, m] so the DMA is contiguous per partition.
    arr = wsign.reshape(CB, P, CB, P, 3, 3).transpose(0, 4, 5, 2, 3, 1)
    wb = np.ascontiguousarray(arr.reshape(CB * 9 * CB, P, P).transpose(1, 0, 2))

    pp = np.empty((P, CB, 3), dtype=np.float32)
    for cb in range(CB):
        ch = slice(cb * P, (cb + 1) * P)
        pp[:, cb, 0] = scale[ch] * scale[ch]
        pp[:, cb, 1] = gamma[ch] * scale[ch]
        pp[:, cb, 2] = beta[ch]
    return wb, pp


def kernel(x, w, scale, gamma, beta):
    global _CACHED
    if _CACHED is None:
        _CACHED = _build_nc()
    nc = _CACHED

    x = np.asarray(x, dtype=np.float32)
    wb, pp = _prep_shared(w, scale, gamma, beta)

    in_maps = []
    for i in range(N_CORES):
        xs = x[i * N_PER_CORE:(i + 1) * N_PER_CORE]
        xs = np.ascontiguousarray(xs.reshape(N_PER_CORE, CB, P, HW))
        in_maps.append({"x": xs, "wb": wb, "pp": pp})

    trace = bool(int(os.environ.get("KERNEL_TRACE", "0")))
    kw = {}
    tdir = os.environ.get("KERNEL_TRACE_DIR")
    if trace and tdir:
        os.makedirs(tdir, exist_ok=True)
        kw["tmpdir"] = tdir
    res = run_bass_kernel_spmd(nc, in_maps, core_ids=list(range(N_CORES)),
                               trace=trace, **kw)
    if trace:
        import kernel as _self
        _self.LAST_EXEC_NS = res.exec_time_ns
        _self.LAST_RESULTS = res

    out = np.empty((64, C, H, W), dtype=np.float32)
    for i in range(N_CORES):
        o = res.results[i]["out"].reshape(N_PER_CORE, C, H, W)
        out[i * N_PER_CORE:(i + 1) * N_PER_CORE] = o
    return out
